# revision 1
# baseline (speedup 1.0000x reference)
"""CondConv2d (MoE-routed 3x3 conv) Trainium2 Bass kernel.

Full-input contract: kernel(**inputs) takes the unsharded tensors and
returns the full [32, 192, 56, 56] output. Internally: data-parallel
across batch over 8 NeuronCores (4 samples per core). Each core mixes
its own per-sample weights on-device (routing @ expert_weights via
chained DVE multiply-accumulate; experts replicated in SBUF) and runs
its samples' convolutions as shifted float32r matmuls accumulating in
PSUM; PSUM is drained by the scalar engine with the routed bias fused
into the copy, keeping the vector engine free for mixing.

Conv decomposition per sample (I=O=192, K=3, H=W=56, pad=1):
  out[o, p] = sum over (i, dy, dx) of w[o, i, dy, dx] * xpad[i, h+dy, w+dx]
As matmuls with contraction on the SBUF partition dim (<=128):
  - channels i in [0,128): 9 chunks (one per (dy,dx)), K=128
  - channels i in [128,192): stored twice in one tile, second copy
    pre-shifted one row, so a single K=128 matmul covers (dy=0, dy=1)
    for a given dx -> 3 paired chunks; dy=2 is 3 more K=64 chunks.
  => 15 accumulating matmuls per (O-chunk, pixel-tile); O split 128+64.
Pixels tiled 7 x 448 (8 rows of 56), each tile in its own PSUM bank.

Weights arrive offset-group-major so sample 0's first head chunks can
start as soon as the first group's experts are mixed; mixing and x DMA
for sample b+1 are emitted ahead of sample b's conv so the DVE stream
runs a full sample ahead of the PE.
"""

import numpy as np

B, E = 32, 8
O, I = 192, 192
H, W = 56, 56
HP = H + 2  # padded side
NCORES = 8
BPC = B // NCORES  # samples per core
NT = 7  # pixel tiles per sample
RPT = 8  # output rows per pixel tile
TW = RPT * W  # 448 pixels per tile
NCH = 15  # accumulating matmul chunks per (O-chunk, pixel-tile)
OC = ((0, 128), (128, 64))  # (o_start, o_size) chunks

_CACHE = {}


def _build():
    import concourse.bass as bass  # noqa: F401
    from concourse import bacc, mybir, tile

    dt = mybir.dt
    f32 = dt.float32
    f32r = dt.float32r
    MULT = mybir.AluOpType.mult
    ADD = mybir.AluOpType.add
    IDENT = mybir.ActivationFunctionType.Identity

    nc = bacc.Bacc(
        "TRN2",
        target_bir_lowering=False,
        debug=False,
        enable_asserts=False,
        num_devices=NCORES,
    )

    xin = nc.dram_tensor("xin", [BPC, I, H, W], f32, kind="ExternalInput").ap()
    # wht free layout: ((g*E + e)*3 + d)*O + o with off = g*3 + d
    wht_d = nc.dram_tensor("wht", [128, 9 * E * O], f32, kind="ExternalInput").ap()
    # wtp/wt2 free layout: e*(3*O) + dx*O + o
    wtp_d = nc.dram_tensor("wtp", [128, 3 * E * O], f32, kind="ExternalInput").ap()
    wt2_d = nc.dram_tensor("wt2", [64, 3 * E * O], f32, kind="ExternalInput").ap()
    bias_d = nc.dram_tensor("bias", [E, O], f32, kind="ExternalInput").ap()
    rt_d = nc.dram_tensor("rt", [E, BPC], f32, kind="ExternalInput").ap()
    rf_d = nc.dram_tensor("rf", [1, BPC * E], f32, kind="ExternalInput").ap()
    out_d = nc.dram_tensor("out", [BPC, O, H * W], f32, kind="ExternalOutput").ap()

    with tile.TileContext(nc) as tc:
        with (
            tc.tile_pool(name="consts", bufs=1) as consts,
            tc.tile_pool(name="wm", bufs=3) as wm_pool,
            tc.tile_pool(name="stage", bufs=4) as stage_pool,
            tc.tile_pool(name="psum1", bufs=1, space="PSUM") as psum1,
            tc.tile_pool(name="cpsum", bufs=1, space="PSUM") as cpsum,
        ):
            # ---- small inputs (needed for broadcast/bias matmuls)
            bias_sb = consts.tile([E, O], f32)
            nc.sync.dma_start(out=bias_sb, in_=bias_d)
            rt_sb = consts.tile([E, BPC], f32)
            nc.sync.dma_start(out=rt_sb, in_=rt_d)
            rf_sb = consts.tile([1, BPC * E], f32)
            nc.sync.dma_start(out=rf_sb, in_=rf_d)

            # ---- broadcast routing to all partitions + mix bias, via matmuls
            ones_sb = consts.tile([1, 128], f32)
            nc.vector.memset(ones_sb, 1.0)
            NRB = BPC * E
            ps0 = psum1.tile([128, 2 * BPC + NRB], f32)  # one PSUM bank
            nc.tensor.matmul(ps0[:, 0:BPC], lhsT=bias_sb[:, 0:128], rhs=rt_sb,
                             start=True, stop=True)
            nc.tensor.matmul(ps0[0:64, BPC:2 * BPC], lhsT=bias_sb[:, 128:192],
                             rhs=rt_sb, start=True, stop=True)
            nc.tensor.matmul(ps0[:, 2 * BPC:], lhsT=ones_sb, rhs=rf_sb,
                             start=True, stop=True)
            bias_cols = consts.tile([128, 2 * BPC], f32)
            nc.vector.tensor_copy(bias_cols[:, 0:BPC], ps0[:, 0:BPC])
            nc.vector.tensor_copy(bias_cols[0:64, BPC:2 * BPC], ps0[0:64, BPC:2 * BPC])
            rb = consts.tile([128, NRB], f32)
            nc.vector.tensor_copy(rb, ps0[:, 2 * BPC:])

            # ---- persistent padded-x tiles first: sample 0's x must not
            # queue behind 10.6MB of weight DMA.
            xh = [consts.tile([128, HP, HP], f32r, tag=f"xh{i}", name=f"xh{i}")
                  for i in range(2)]
            xt = [consts.tile([128, HP, HP], f32r, tag=f"xt{i}", name=f"xt{i}")
                  for i in range(2)]
            for t_ in xh + xt:
                nc.vector.memset(t_[:, 0, :].bitcast(f32), 0.0)
                nc.vector.memset(t_[:, HP - 1, :].bitcast(f32), 0.0)
                nc.vector.memset(t_[:, :, 0].bitcast(f32), 0.0)
                nc.vector.memset(t_[:, :, HP - 1].bitcast(f32), 0.0)

            def emit_x_dma(b):
                xhb, xtb = xh[b % 2], xt[b % 2]
                nc.sync.dma_start(out=xhb[:, 1:H + 1, 1:W + 1],
                                  in_=xin[b, 0:128].bitcast(f32r))
                nc.sync.dma_start(out=xtb[0:64, 1:H + 1, 1:W + 1],
                                  in_=xin[b, 128:192].bitcast(f32r))
                nc.sync.dma_start(out=xtb[64:128, 0:H, 1:W + 1],
                                  in_=xin[b, 128:192].bitcast(f32r))

            emit_x_dma(0)

            # ---- resident expert weights, offset-group-major arrival order
            wht = consts.tile([128, 3, 3 * E * O], f32)  # [i, g, (e, d, o)]
            wtp = consts.tile([128, E, 3 * O], f32)  # [i2pair, e, (dx, o)]
            wt2 = consts.tile([64, E, 3 * O], f32)
            for g in range(3):
                for e in range(E):
                    base = (g * E + e) * (3 * O)
                    nc.sync.dma_start(out=wht[:, g, e * 3 * O:(e + 1) * 3 * O],
                                      in_=wht_d[:, base:base + 3 * O])
            for e in range(E):
                nc.sync.dma_start(out=wtp[:, e, :],
                                  in_=wtp_d[:, e * 3 * O:(e + 1) * 3 * O])
                nc.sync.dma_start(out=wt2[:, e, :],
                                  in_=wt2_d[:, e * 3 * O:(e + 1) * 3 * O])

            def emit_mix_head(b):
                # wmh free = off*O + o
                wmh = wm_pool.tile([128, 9 * O], f32r, tag="wmh", name="wmh")
                for g in range(3):  # independent chains per offset group
                    dst = wmh[:, g * 3 * O:(g + 1) * 3 * O]
                    for e in range(E):
                        rc = rb[:, b * E + e:b * E + e + 1]
                        srcw = wht[:, g, e * 3 * O:(e + 1) * 3 * O]
                        if e == 0:
                            nc.vector.tensor_scalar_mul(dst, srcw, rc)
                        else:
                            nc.vector.scalar_tensor_tensor(
                                dst, srcw, rc, dst, op0=MULT, op1=ADD)
                return wmh

            def emit_mix_tail(b):
                # wmp/wm2 free = dx*O + o
                wmp = wm_pool.tile([128, 3 * O], f32r, tag="wmp", name="wmp")
                wm2 = wm_pool.tile([64, 3 * O], f32r, tag="wm2", name="wm2")
                for e in range(E):
                    rc = rb[:, b * E + e:b * E + e + 1]
                    rc64 = rb[0:64, b * E + e:b * E + e + 1]
                    if e == 0:
                        nc.vector.tensor_scalar_mul(wmp, wtp[:, 0, :], rc)
                        nc.vector.tensor_scalar_mul(wm2, wt2[:, 0, :], rc64)
                    else:
                        nc.vector.scalar_tensor_tensor(
                            wmp, wtp[:, e, :], rc, wmp, op0=MULT, op1=ADD)
                        nc.vector.scalar_tensor_tensor(
                            wm2, wt2[:, e, :], rc64, wm2, op0=MULT, op1=ADD)
                return wmp, wm2

            def emit_mix(b):
                return (emit_mix_head(b), *emit_mix_tail(b))

            wm = {0: emit_mix(0)}

            for b in range(BPC):
                if b + 1 < BPC:
                    emit_x_dma(b + 1)
                    wm[b + 1] = emit_mix(b + 1)
                xhb, xtb = xh[b % 2], xt[b % 2]
                wmh, wmp, wm2 = wm.pop(b)

                for oci, (o0, osz) in enumerate(OC):
                    pst = [cpsum.tile([128, 512], f32, tag=f"cps{t}",
                                      name=f"cps{t}") for t in range(NT)]
                    ci = 0
                    # head: channels 0:128, one chunk per (dy, dx), K=128
                    for dy in range(3):
                        for dx in range(3):
                            off = dy * 3 + dx
                            lhsT = wmh[:, off * O + o0:off * O + o0 + osz]
                            for t in range(NT):
                                nc.tensor.matmul(
                                    pst[t][0:osz, 0:TW],
                                    lhsT=lhsT,
                                    rhs=xhb[:, t * RPT + dy:t * RPT + dy + RPT,
                                            dx:dx + W],
                                    start=(ci == 0), stop=(ci == NCH - 1))
                            ci += 1
                    # tail paired: channels 128:192, (dy=0,1) pairs, K=128
                    for dx in range(3):
                        lhsT = wmp[:, dx * O + o0:dx * O + o0 + osz]
                        for t in range(NT):
                            nc.tensor.matmul(
                                pst[t][0:osz, 0:TW],
                                lhsT=lhsT,
                                rhs=xtb[:, t * RPT:t * RPT + RPT, dx:dx + W],
                                start=(ci == 0), stop=(ci == NCH - 1))
                        ci += 1
                    # tail dy=2: channels 128:192, K=64
                    for dx in range(3):
                        lhsT = wm2[0:64, dx * O + o0:dx * O + o0 + osz]
                        for t in range(NT):
                            nc.tensor.matmul(
                                pst[t][0:osz, 0:TW],
                                lhsT=lhsT,
                                rhs=xtb[0:64, t * RPT + 2:t * RPT + 2 + RPT,
                                        dx:dx + W],
                                start=(ci == 0), stop=(ci == NCH - 1))
                        ci += 1
                    assert ci == NCH

                    # ---- drain PSUM -> SBUF on ScalarE with fused bias
                    bc = (bias_cols[:, b:b + 1] if osz == 128
                          else bias_cols[0:64, BPC + b:BPC + b + 1])
                    for t in range(NT):
                        st = stage_pool.tile([128, TW], f32, tag="st", name="st")
                        nc.scalar.activation(
                            st[0:osz], pst[t][0:osz, 0:TW], IDENT, bias=bc)
                        nc.sync.dma_start(
                            out=out_d[b, o0:o0 + osz, t * TW:(t + 1) * TW],
                            in_=st[0:osz])

    nc.compile()
    return nc


def _prep_inputs(x, routing_weights, weight, bias):
    x = np.asarray(x, np.float32)
    routing = np.asarray(routing_weights, np.float32)
    weight = np.asarray(weight, np.float32)
    bias = np.asarray(bias, np.float32)

    W5 = weight.reshape(E, O, I, 3, 3)
    # head: [i, dy(g), e, dx(d), o] -> f = ((g*E + e)*3 + d)*O + o
    wht_h = np.ascontiguousarray(
        W5[:, :, :128].transpose(2, 3, 0, 4, 1)).reshape(128, 9 * E * O)
    # tail pair: p<64 -> (i=128+p, dy=0); p>=64 -> (i=64+p, dy=1)
    # f = e*(3*O) + dx*O + o
    t0 = W5[:, :, 128:, 0, :].transpose(2, 0, 3, 1)  # [i2, e, dx, o]
    t1 = W5[:, :, 128:, 1, :].transpose(2, 0, 3, 1)
    wtp_h = np.ascontiguousarray(
        np.concatenate([t0, t1], axis=0)).reshape(128, 3 * E * O)
    wt2_h = np.ascontiguousarray(
        W5[:, :, 128:, 2, :].transpose(2, 0, 3, 1)).reshape(64, 3 * E * O)

    in_maps = []
    for c in range(NCORES):
        sl = slice(c * BPC, (c + 1) * BPC)
        in_maps.append({
            "xin": np.ascontiguousarray(x[sl]),
            "wht": wht_h,
            "wtp": wtp_h,
            "wt2": wt2_h,
            "bias": bias,
            "rt": np.ascontiguousarray(routing[sl].T),
            "rf": np.ascontiguousarray(routing[sl].reshape(1, BPC * E)),
        })
    return in_maps


def _run(in_maps, **kwargs):
    from concourse import bass_utils
    if "nc" not in _CACHE:
        _CACHE["nc"] = _build()
    return bass_utils.run_bass_kernel_spmd(
        _CACHE["nc"], in_maps, core_ids=list(range(NCORES)), **kwargs)


def kernel(x, routing_weights, weight, bias):
    in_maps = _prep_inputs(x, routing_weights, weight, bias)
    res = _run(in_maps)
    out = np.stack([res.results[c]["out"] for c in range(NCORES)], axis=0)
    return out.reshape(B, O, H, W)



# revision 8
# speedup vs baseline: 1.1589x; 1.1589x over previous
"""CondConv2d (MoE-routed 3x3 conv) Trainium2 Bass kernel, v2.

Full-input contract: kernel(**inputs) takes the unsharded tensors and
returns the full [32, 192, 56, 56] output. Data-parallel across batch
over 8 NeuronCores (4 samples per core); each core mixes its own
per-sample weights on-device and runs its samples' convolutions.

v2 layout (transposed matmul orientation, bf16):
  out[p, o] = sum over (i, dy, dx) of xf[i, p + d(dy,dx)] * w[o, i, dy, dx]
with x stored flat 58-col-padded ([128, 3480] per tile, host-prepadded)
so every conv offset is a shifted 1D view: d(dy,dx) = (dy-1)*58+(dx-1).
Each matmul: lhsT = x-view [K<=128 chans, M=128 flat pixels], rhs =
mixed weights [K, N=192 outchans], accumulating in a [128, 192] f32
PSUM tile. bf16 runs at 1 cycle/row (fp32r would be 4x at N<256), so a
tile costs 14 matmuls x 192 rows. Contraction packing per tile:
  - head chans 0:128: 9 chunks, one per (dy, dx), K=128
  - tail chans 128:192 twice per tile: (A; B=A shifted one row) covers
    (dy0, dy1) pairs per dx -> 3 chunks K=128; (A; D=A shifted one col)
    covers (dy2,dx0)+(dy2,dx1) in one K=128 chunk; (A; ones) covers
    (dy2,dx2) with K=65 where partition 64 = ones row x rhs bias row,
    folding the routed bias into the accumulation for free.
  => 14 matmuls per 128-pixel tile, 26 tiles per sample.
Mixing (routing @ experts) is chained scalar_tensor_tensor, split
across DVE (head groups) and Pool (tail tiles) so neither exceeds the
PE's ~29us/sample; PSUM is drained by the scalar engine to bf16 and
host re-strides [3328, 192] -> [192, 56, 56] f32 per sample.
"""

import numpy as np

B, E = 32, 8
O, I = 192, 192
H, W = 56, 56
NCORES = 8
BPC = B // NCORES  # samples per core
FP = 3480  # flat padded x length per channel (58*58=3364 rounded up)
NT = 26  # 128-pixel tiles per sample (covers flat 59..3387)
P0 = 59  # first valid out position in padded-flat coords
NPS = 7  # PSUM tiles in flight (8 banks minus one for the rb matmul)

_CACHE = {}


def _build():
    import concourse.bass as bass  # noqa: F401
    from concourse import bacc, mybir, tile

    dt = mybir.dt
    f32 = dt.float32
    bf16 = dt.bfloat16
    MULT = mybir.AluOpType.mult
    ADD = mybir.AluOpType.add
    IDENT = mybir.ActivationFunctionType.Identity

    nc = bacc.Bacc(
        "TRN2",
        target_bir_lowering=False,
        debug=False,
        enable_asserts=False,
        num_devices=NCORES,
    )

    # x, host-prepadded flat bf16 per sample
    xh_d = nc.dram_tensor("xh", [BPC, 128, FP], bf16, kind="ExternalInput").ap()
    xtp_d = nc.dram_tensor("xtp", [BPC, 128, FP], bf16, kind="ExternalInput").ap()
    xtd_d = nc.dram_tensor("xtd", [BPC, 128, FP], bf16, kind="ExternalInput").ap()
    xt2_d = nc.dram_tensor("xt2", [BPC, 65, FP], bf16, kind="ExternalInput").ap()
    # weights: wht free = ((dy*E + e)*3 + dx)*O + o
    wht_d = nc.dram_tensor("wht", [128, 9 * E * O], bf16, kind="ExternalInput").ap()
    # fused tail consts, free = e*960 + c with c: 0:576 pair (A: dy0,
    # B: dy1), 576:768 colpair (A: dy2dx0, D: dy2dx1), 768:960 dy2dx2
    # on partitions 0:64 with bias[e] on partition 64, zeros above
    wtt_d = nc.dram_tensor("wtt", [128, E * 960], bf16, kind="ExternalInput").ap()
    rf_d = nc.dram_tensor("rf", [1, BPC * E], f32, kind="ExternalInput").ap()
    out_d = nc.dram_tensor("out", [BPC, NT * 128, O], bf16, kind="ExternalOutput").ap()

    # flat-view offset for conv tap (dy, dx)
    def dlt(dy, dx):
        return (dy - 1) * 58 + (dx - 1)

    with tile.TileContext(nc) as tc:
        with (
            tc.tile_pool(name="consts", bufs=1) as consts,
            tc.tile_pool(name="xp", bufs=2) as xp,
            tc.tile_pool(name="wm", bufs=3) as wm_pool,
            tc.tile_pool(name="stage", bufs=6) as stage_pool,
            tc.tile_pool(name="psum1", bufs=1, space="PSUM") as psum1,
            tc.tile_pool(name="cpsum", bufs=1, space="PSUM") as cpsum,
        ):
            # ---- routing broadcast to all partitions via K=1 ones matmul
            rf_sb = consts.tile([1, BPC * E], f32)
            nc.sync.dma_start(out=rf_sb, in_=rf_d)
            ones_sb = consts.tile([1, 128], f32)
            nc.vector.memset(ones_sb, 1.0)
            ps0 = psum1.tile([128, BPC * E], f32)
            nc.tensor.matmul(ps0, lhsT=ones_sb, rhs=rf_sb, start=True, stop=True)
            rb = consts.tile([128, BPC * E], f32)
            nc.vector.tensor_copy(rb, ps0)

            # ---- x DMA for sample 0 ahead of the big weight DMAs
            def emit_x_dma(b):
                xhb = xp.tile([128, FP], bf16, tag="xh", name="xh")
                xtpb = xp.tile([128, FP], bf16, tag="xtp", name="xtp")
                xtdb = xp.tile([128, FP], bf16, tag="xtd", name="xtd")
                xt2b = xp.tile([65, FP], bf16, tag="xt2", name="xt2")
                nc.sync.dma_start(out=xhb, in_=xh_d[b])
                nc.sync.dma_start(out=xtpb, in_=xtp_d[b])
                nc.sync.dma_start(out=xtdb, in_=xtd_d[b])
                nc.sync.dma_start(out=xt2b, in_=xt2_d[b])
                return xhb, xtpb, xtdb, xt2b

            xt = {0: emit_x_dma(0)}

            # ---- resident expert weights, head-group-major arrival order
            wht = consts.tile([128, 3, E, 3 * O], bf16)  # [i, dy, e, (dx, o)]
            wtt = consts.tile([128, E, 960], bf16)  # fused tail consts
            for dy in range(3):
                for e in range(E):
                    base = ((dy * E + e) * 3) * O
                    nc.sync.dma_start(out=wht[:, dy, e, :],
                                      in_=wht_d[:, base:base + 3 * O])
            for e in range(E):
                nc.sync.dma_start(out=wtt[:, e, :],
                                  in_=wtt_d[:, e * 960:(e + 1) * 960])

            def emit_mix(b):
                # head: chained scalar_tensor_tensor MAC on DVE, one 3D op
                # per expert; tail: ACT does tmp = W_e * r (per-partition
                # scale), Pool accumulates wmt += tmp (TensorScalarPtr is
                # not legal on Pool, tensor_tensor is).
                wmh = wm_pool.tile([128, 3, 3 * O], bf16, tag="wmh", name="wmh")
                wmt = wm_pool.tile([128, 960], bf16, tag="wmt", name="wmt")
                for e in range(E):
                    rc = rb[:, b * E + e:b * E + e + 1]
                    src = wht[:, :, e, :]
                    if e == 0:
                        nc.vector.tensor_scalar_mul(wmh, src, rc)
                    else:
                        nc.vector.scalar_tensor_tensor(
                            wmh, src, rc, wmh, op0=MULT, op1=ADD)
                for e in range(E):
                    rc = rb[:, b * E + e:b * E + e + 1]
                    if e == 0:
                        nc.scalar.activation(wmt, wtt[:, 0, :], IDENT, scale=rc)
                    else:
                        tmp = wm_pool.tile([128, 960], bf16, tag="tmt",
                                           name="tmt")
                        nc.scalar.activation(tmp, wtt[:, e, :], IDENT, scale=rc)
                        nc.gpsimd.tensor_tensor(wmt, wmt, tmp, op=ADD)
                return wmh, wmt

            wm = {0: emit_mix(0)}

            for b in range(BPC):
                if b + 1 < BPC:
                    xt[b + 1] = emit_x_dma(b + 1)
                    wm[b + 1] = emit_mix(b + 1)
                xhb, xtpb, xtdb, xt2b = xt.pop(b)
                wmh, wmt = wm.pop(b)

                for t in range(NT):
                    ps = cpsum.tile([128, O], f32, tag=f"cps{t % NPS}",
                                    name=f"cps{t % NPS}")
                    s0 = P0 + t * 128
                    ci = 0
                    # head: one K=128 chunk per (dy, dx)
                    for dy in range(3):
                        for dx in range(3):
                            s = s0 + dlt(dy, dx)
                            nc.tensor.matmul(
                                ps, lhsT=xhb[:, s:s + 128],
                                rhs=wmh[:, dy, dx * O:(dx + 1) * O],
                                start=(ci == 0), stop=False)
                            ci += 1
                    # tail (dy0, dy1) pairs per dx
                    for dx in range(3):
                        s = s0 + dlt(0, dx)
                        nc.tensor.matmul(
                            ps, lhsT=xtpb[:, s:s + 128],
                            rhs=wmt[:, dx * O:(dx + 1) * O],
                            start=False, stop=False)
                        ci += 1
                    # tail (dy2, dx0)+(dy2, dx1) via col-shifted pair
                    s = s0 + dlt(2, 0)
                    nc.tensor.matmul(ps, lhsT=xtdb[:, s:s + 128],
                                     rhs=wmt[:, 576:768],
                                     start=False, stop=False)
                    # tail (dy2, dx2) K=64 + bias via ones row, K=65
                    s = s0 + dlt(2, 2)
                    nc.tensor.matmul(ps, lhsT=xt2b[0:65, s:s + 128],
                                     rhs=wmt[0:65, 768:960],
                                     start=False, stop=True)

                    st = stage_pool.tile([128, O], bf16, tag="st", name="st")
                    nc.scalar.activation(st, ps, IDENT)
                    nc.sync.dma_start(out=out_d[b, t * 128:(t + 1) * 128, :],
                                      in_=st)

    nc.compile()
    return nc


def _prep_inputs(x, routing_weights, weight, bias):
    import ml_dtypes

    bf = ml_dtypes.bfloat16
    x = np.asarray(x, np.float32)
    routing = np.asarray(routing_weights, np.float32)
    weight = np.asarray(weight, np.float32)
    bias = np.asarray(bias, np.float32)

    W5 = weight.reshape(E, O, I, 3, 3)
    # head: [i, dy, e, dx, o]
    wht_h = np.ascontiguousarray(
        W5[:, :, :128].transpose(2, 3, 0, 4, 1)).reshape(128, 9 * E * O)
    # fused tail consts [128, e, 960]:
    #   cols 0:576  pair (partitions A = tail dy0, B = tail dy1), (dx, o)
    #   cols 576:768 colpair (A = dy2dx0, D = dy2dx1)
    #   cols 768:960 dy2dx2 on partitions 0:64, bias on partition 64
    wtt_h = np.zeros((128, E, 960), np.float32)
    t0 = W5[:, :, 128:, 0, :].transpose(2, 0, 3, 1).reshape(64, E, 576)
    t1 = W5[:, :, 128:, 1, :].transpose(2, 0, 3, 1).reshape(64, E, 576)
    wtt_h[:64, :, 0:576] = t0
    wtt_h[64:, :, 0:576] = t1
    wtt_h[:64, :, 576:768] = W5[:, :, 128:, 2, 0].transpose(2, 0, 1)
    wtt_h[64:, :, 576:768] = W5[:, :, 128:, 2, 1].transpose(2, 0, 1)
    wtt_h[:64, :, 768:960] = W5[:, :, 128:, 2, 2].transpose(2, 0, 1)
    wtt_h[64, :, 768:960] = bias

    wht_b = wht_h.astype(bf)
    wtt_b = wtt_h.reshape(128, E * 960).astype(bf)

    # flat 58-padded x (+ room for view overhang), bf16
    xf = np.zeros((B, I, 60, 58), np.float32)
    xf[:, :, 1:57, 1:57] = x
    xf = xf.reshape(B, I, FP).astype(bf)
    xh_h = np.ascontiguousarray(xf[:, :128])  # [B, 128, FP]
    A = xf[:, 128:]  # [B, 64, FP] tail, unshifted
    sh58 = np.zeros_like(A)
    sh58[:, :, :FP - 58] = A[:, :, 58:]
    sh1 = np.zeros_like(A)
    sh1[:, :, :FP - 1] = A[:, :, 1:]
    xtp_h = np.ascontiguousarray(np.concatenate([A, sh58], axis=1))
    xtd_h = np.ascontiguousarray(np.concatenate([A, sh1], axis=1))
    ones_row = np.ones((B, 1, FP), np.float32).astype(bf)
    xt2_h = np.ascontiguousarray(np.concatenate([A, ones_row], axis=1))

    in_maps = []
    for c in range(NCORES):
        sl = slice(c * BPC, (c + 1) * BPC)
        in_maps.append({
            "xh": xh_h[sl],
            "xtp": xtp_h[sl],
            "xtd": xtd_h[sl],
            "xt2": xt2_h[sl],
            "wht": wht_b,
            "wtt": wtt_b,
            "rf": np.ascontiguousarray(routing[sl].reshape(1, BPC * E)),
        })
    return in_maps


def _run(in_maps, **kwargs):
    from concourse import bass_utils
    if "nc" not in _CACHE:
        _CACHE["nc"] = _build()
    return bass_utils.run_bass_kernel_spmd(
        _CACHE["nc"], in_maps, core_ids=list(range(NCORES)), **kwargs)


def kernel(x, routing_weights, weight, bias):
    in_maps = _prep_inputs(x, routing_weights, weight, bias)
    res = _run(in_maps)
    out = np.empty((B, O, H, W), np.float32)
    for c in range(NCORES):
        arr = np.asarray(res.results[c]["out"]).astype(np.float32)
        # rows j of arr map to padded-flat position P0 + j; out pixel
        # (r, cc) lives at j = 58*r + cc
        v = arr[:, :3248].reshape(BPC, 56, 58, O)[:, :, :56]
        out[c * BPC:(c + 1) * BPC] = v.transpose(0, 3, 1, 2)
    return out
